# revision 53
# baseline (speedup 1.0000x reference)
"""Trainium2 Bass kernel for nn_BigramHash: out = tab[hash(t,prev)] @ w_proj.T.

Strategy (v2 — SBUF-resident bf16 table + dma_gather):
  - Table rows are sharded across the 8 cores (384 rows each); each core
    folds its slice on-device: tab2 = tab[rows] @ w_proj.T, computed with
    fp32r matmuls (full-rate on the PE at near-fp32 precision), and keeps
    the result in SBUF as bf16 (0.77 MB) — the gather never touches HBM.
  - The host routes each token to the core owning its hashed row.  Tokens
    are ordered by table-row chunk (so early gather groups depend only on
    early fold chunks) and, within a chunk, round-robined across the row's
    SBUF partition (r % 128) in an AXI-port-interleaved order so the
    SBUF-source gather reads spread across all 16 SBUF ports.
  - Each core recomputes the bigram hash for its tokens on DVE (exact in
    fp32), rebases it into its local slice, and emits int16 indices in the
    dma_gather layout ([16, n/16] blocks replicated across the 8 Q7-core
    partition groups — the replication comes free by replicating the raw
    t/prev inputs host-side).
  - dma_gather (SBUF-source, transpose mode) expands the table rows to
    token order: a handful of instructions replace the 33 indirect DMAs +
    16.9 MB HBM round-trip of v1.  Output tiles [128, 8, gsz] are streamed
    to DRAM as bf16 (half the write traffic of f32).
  - The host de-transposes, upcasts to f32, and scatters rows back to
    token order.

Per-core HBM traffic: ~5.8 MB loads + 8.7 MB output writes (vs ~40 MB in
v1).  Gather traffic rides the SBUF fabric instead of HBM.

Accuracy: the only rounding vs the fp32 reference is fp32r's reduced
multiplier precision in the fold plus one bf16 quantization of tab2 —
elementwise relative error ~2^-9.
"""

import numpy as np

import concourse.bass as bass
import concourse.tile as tile
from concourse import bacc, mybir
from concourse.bass_utils import run_bass_kernel_spmd

N_CORES = 8
B, T = 4, 8192
SZ, D = 3072, 1024
NTOK = B * T                      # 32768
SLICE = SZ // N_CORES             # 384 table rows per core
RC_LOC = SLICE // 128             # 3 fold row-chunks per core
KC = D // 128                     # 8 contraction chunks

C_T = 31337 % SZ                  # 617
C_P = 1000003 % SZ                # 1603

GROUP_TOKENS = 512                # tokens per gather group (multiple of 128)
GATHER_SRC = "dram"               # "dram" (non-transpose) | "sbuf" (transpose)
FOLD_DTYPE = "bf16"               # "f32r" | "f32" | "bf16"

_CACHE = {}


def declare_io(nc, tiles, fold_dtype="f32r"):
    f32 = mybir.dt.float32
    i32 = mybir.dt.int32
    bf16 = mybir.dt.bfloat16
    mm_dt = {"f32r": mybir.dt.float32r, "f32": f32, "bf16": bf16}[fold_dtype]
    s = tiles * 128 // 16          # columns of the [16-wrap, s] token layout
    # t|prev packed side by side: cols [0,s) = t, [s,2s) = prev
    t_ap = nc.dram_tensor("t_sh", [128, 2 * s], i32, kind="ExternalInput").ap()
    base_ap = nc.dram_tensor("base", [128, 1], f32, kind="ExternalInput").ap()
    # tabT blocked by fold row-chunk: [p, rc*KC*128 + kc*128 + m]
    #   = tab[rc*128 + m? no: = w/tab transpose, see make_in_maps]
    tabT_ap = nc.dram_tensor(
        "tabT", [128, RC_LOC * KC * 128], mm_dt, kind="ExternalInput"
    ).ap()
    # wT packed: [p, kc*D + e] = w_proj.T[kc*128 + p, e]
    wT_ap = nc.dram_tensor(
        "w_projT", [128, KC * D], mm_dt, kind="ExternalInput"
    ).ap()
    out_ap = nc.dram_tensor(
        "out_sh", [128, 8 * tiles * 128], bf16, kind="ExternalOutput"
    ).ap()
    tab2_ap = nc.dram_tensor("tab2", [SLICE, D], bf16).ap()
    return t_ap, base_ap, tabT_ap, wT_ap, out_ap, tab2_ap


def emit_body(nc, tc, io, tiles, groups, fold_dtype=FOLD_DTYPE,
              do_hash=True, do_fold=True, do_gather=True, do_out=True,
              gather_src=GATHER_SRC, single_packet=True):
    f32 = mybir.dt.float32
    i32 = mybir.dt.int32
    i16 = mybir.dt.int16
    bf16 = mybir.dt.bfloat16
    t_ap, base_ap, tabT_ap, wT_ap, out_ap, tab2_ap = io
    cap = tiles * 128
    s = cap // 16
    sd = tiles
    with (
        tc.tile_pool(name="weights", bufs=1) as wpool,
        tc.tile_pool(name="psum", bufs=2, space="PSUM") as ppool,
        tc.tile_pool(name="idx", bufs=1) as ipool,
        tc.tile_pool(name="gather", bufs=4) as gpool,
    ):
        # ---- small loads first: hash inputs ----
        t_sb = ipool.tile([128, 2 * s], i32)
        nc.scalar.dma_start(t_sb[:], t_ap[:])
        base_sb = ipool.tile([128, 1], f32)
        nc.scalar.dma_start(base_sb[:], base_ap[:])

        # ---- big loads: w_projT first (gates every fold chunk), then the
        # tabT slice in per-fold-chunk blocks so fold chunk 0 can start as
        # soon as its third of tabT lands.  Layouts are packed host-side so
        # each load is one DMA with >=2KB-per-partition descriptors. ----
        mm_dt = {"f32r": mybir.dt.float32r, "f32": f32, "bf16": bf16}[fold_dtype]
        wT_sb = wpool.tile([128, KC * D], mm_dt, tag="wT")
        nc.scalar.dma_start(wT_sb[:], wT_ap[:])
        tabT_sb = wpool.tile([128, RC_LOC * KC * 128], mm_dt, tag="tabT")
        for rc in range(RC_LOC):
            blk = slice(rc * KC * 128, (rc + 1) * KC * 128)
            nc.scalar.dma_start(tabT_sb[:, blk], tabT_ap[:, blk])

        # ---- hash on DVE in one full-width pass (exact in fp32; all
        # intermediates < 2^24).  Token i lives at [16g + i%16, i//16] for
        # every Q7 partition group g (inputs replicated host-side, so the
        # result lands replicated for free). ----
        idx_sb = ipool.tile([128, s], i16)
        tpf = ipool.tile([128, 2 * s], f32)
        tpm = ipool.tile([128, 2 * s], f32)
        m_t = ipool.tile([128, 2 * s], f32, tag="mod_m")
        qi_t = ipool.tile([128, 2 * s], i32, tag="mod_qi")
        sacc = ipool.tile([128, s], f32)

        def mod_sz(dst, src, lo, w):
            m, qi = m_t[:, lo : lo + w], qi_t[:, lo : lo + w]
            nc.vector.tensor_scalar(m, src, 1.0 / SZ, None,
                                    op0=mybir.AluOpType.mult)
            nc.vector.tensor_copy(qi, m)
            nc.vector.tensor_copy(m, qi)
            nc.vector.tensor_scalar(m, m, float(SZ), None,
                                    op0=mybir.AluOpType.mult)
            nc.vector.tensor_tensor(dst, src, m, op=mybir.AluOpType.subtract)
            nc.vector.tensor_scalar(m, dst, 0.0, float(SZ),
                                    op0=mybir.AluOpType.is_lt,
                                    op1=mybir.AluOpType.mult)
            nc.vector.tensor_tensor(dst, dst, m, op=mybir.AluOpType.add)
            nc.vector.tensor_scalar(m, dst, float(SZ), float(-SZ),
                                    op0=mybir.AluOpType.is_ge,
                                    op1=mybir.AluOpType.mult)
            nc.vector.tensor_tensor(dst, dst, m, op=mybir.AluOpType.add)

        if do_hash:
            nc.vector.tensor_copy(tpf[:], t_sb[:])          # i32 -> f32
            mod_sz(tpm[:], tpf[:], 0, 2 * s)
            nc.vector.tensor_scalar(tpm[:, 0:s], tpm[:, 0:s], float(C_T),
                                    None, op0=mybir.AluOpType.mult)
            nc.vector.tensor_scalar(tpm[:, s : 2 * s], tpm[:, s : 2 * s],
                                    float(C_P), None, op0=mybir.AluOpType.mult)
            nc.vector.tensor_tensor(sacc[:], tpm[:, 0:s], tpm[:, s : 2 * s],
                                    op=mybir.AluOpType.add)
            sf = tpm[:, 0:s]
            mod_sz(sf, sacc[:], 0, s)
            # rebase into the local slice and clamp (pad slots and foreign
            # rows are discarded by the host)
            nc.vector.tensor_tensor(sf, sf,
                                    base_sb[:, 0:1].to_broadcast([128, s]),
                                    op=mybir.AluOpType.subtract)
            nc.vector.tensor_scalar(sf, sf, 0.0, float(SLICE - 1),
                                    op0=mybir.AluOpType.max,
                                    op1=mybir.AluOpType.min)
            nc.vector.tensor_copy(idx_sb[:], sf)
        else:
            nc.vector.memset(idx_sb[:], 0)

        # ---- fold: tab2 = tab[rows] @ w_proj.T, kept in SBUF as bf16 ----
        tab2_sb = wpool.tile([128, RC_LOC * D], bf16, tag="tab2")
        if not do_fold:
            nc.vector.memset(tab2_sb[:], 0)
        for rc in range(RC_LOC if do_fold else 0):
            ps = ppool.tile([128, D], f32)
            for kc in range(KC):
                lhsT = tabT_sb[:, (rc * KC + kc) * 128 : (rc * KC + kc + 1) * 128]
                nc.tensor.matmul(
                    ps[:, 0:512], lhsT, wT_sb[:, kc * D : kc * D + 512],
                    start=(kc == 0), stop=(kc == KC - 1),
                )
                nc.tensor.matmul(
                    ps[:, 512:1024], lhsT,
                    wT_sb[:, kc * D + 512 : (kc + 1) * D],
                    start=(kc == 0), stop=(kc == KC - 1),
                )
            nc.vector.tensor_copy(tab2_sb[:, rc * D : (rc + 1) * D], ps[:])
            if gather_src in ("dram", "hybrid"):
                nc.sync.dma_start(
                    tab2_ap[rc * 128 : (rc + 1) * 128, :],
                    tab2_sb[:, rc * D : (rc + 1) * D],
                )

        # ---- gather groups: SBUF-source dma_gather + bf16 out stream ----
        for gi, (g0, gsz, span) in enumerate(groups):
            mode = gather_src if gather_src != "hybrid" else (
                "sbuf" if gi % 2 == 1 else "dram")
            if mode == "sbuf":
                dst = gpool.tile([128, 8, gsz], bf16, tag=f"gs{gsz}")
            else:
                dst = gpool.tile([128, gsz // 128, D], bf16, tag=f"gd{gsz}")
            if do_gather:
                if mode == "sbuf":
                    nc.gpsimd.dma_gather(
                        out_ap=dst[:],
                        in_ap=tab2_sb[:, : span * D],
                        idxs_ap=idx_sb[:, g0 // 16 : (g0 + gsz) // 16],
                        num_idxs=gsz,
                        num_idxs_reg=gsz,
                        elem_size=D,
                        transpose=True,
                        sbuf_tokens_per_rank=128,
                        sbuf_free_dim_per_rank=2 * D,
                        single_packet=single_packet,
                    )
                else:
                    nc.gpsimd.dma_gather(
                        out_ap=dst[:],
                        in_ap=tab2_ap[0 : span * 128, :],
                        idxs_ap=idx_sb[:, g0 // 16 : (g0 + gsz) // 16],
                        num_idxs=gsz,
                        num_idxs_reg=gsz,
                        elem_size=D,
                        single_packet=single_packet,
                    )
            elif not do_out:
                continue
            if do_out:
                if mode == "sbuf":
                    src = dst[:].rearrange("p c j -> p (c j)")
                else:
                    src = dst[:].rearrange("p j d -> p (j d)")
                out_eng = nc.sync if gi % 2 == 0 else nc.scalar
                out_eng.dma_start(out_ap[:, 8 * g0 : 8 * (g0 + gsz)], src)


def build(tiles, loop_iters=None, bmax=None, fold_dtype=FOLD_DTYPE, **body_kw):
    """Build the SPMD Bass program (same program for all 8 cores).

    tiles: per-core token capacity in 128-token units.
    bmax: tuple of (g0, gsz, span) gather-group specs (host-derived, maxed
    across cores so the program is identical on every core).
    loop_iters: wrap the idempotent body in a For_i loop (timing only).
    """
    groups = bmax
    key = ("nc", tiles, loop_iters, groups, fold_dtype,
           tuple(sorted(body_kw.items())))
    if key in _CACHE:
        return _CACHE[key]
    nc = bacc.Bacc("TRN2", target_bir_lowering=False, debug=False)
    io = declare_io(nc, tiles, fold_dtype=fold_dtype)
    with tile.TileContext(nc) as tc:
        if loop_iters is None:
            emit_body(nc, tc, io, tiles, groups, fold_dtype=fold_dtype, **body_kw)
        else:
            with tc.For_i(0, loop_iters, 1):
                emit_body(nc, tc, io, tiles, groups, fold_dtype=fold_dtype,
                          **body_kw)
    nc.compile()
    _CACHE[key] = nc
    return nc


def _hash_idx_host(t_flat, p_flat):
    a = (t_flat.astype(np.int64) % SZ) * C_T
    b = (p_flat.astype(np.int64) % SZ) * C_P
    return ((a + b) % SZ).astype(np.int64)


def _port_interleaved_partitions():
    """Permutation of 0..127 such that consecutive entries cycle through
    all 16 SBUF AXI ports (port(p) = 2*((p%32)//4) + (p>=64))."""
    port_of = lambda p: 2 * ((p % 32) // 4) + (1 if p >= 64 else 0)
    by_port = [[] for _ in range(16)]
    for p in range(128):
        by_port[port_of(p)].append(p)
    perm = []
    for k in range(8):
        for q in range(16):
            perm.append(by_port[q][k])
    return np.array(perm, dtype=np.int64)


def route(t):
    """Host routing: owner core + per-core slot order.

    Slot order per core: by fold chunk (so gather groups only depend on a
    prefix of the fold), then round-robin across partitions (row % 128) in
    port-interleaved order to spread SBUF-source gather reads."""
    t = np.asarray(t)
    prev = np.pad(t[:, :-1], ((0, 0), (1, 0)))
    t_flat = np.ascontiguousarray(t, dtype=np.int32).reshape(-1)
    p_flat = np.ascontiguousarray(prev, dtype=np.int32).reshape(-1)
    idx = _hash_idx_host(t_flat, p_flat)
    owner = idx // SLICE
    counts = np.bincount(owner, minlength=N_CORES)
    tiles = max(1, int(-(-counts.max() // 128)))
    cap = tiles * 128

    perm = _port_interleaved_partitions()
    rank_of_part = np.empty(128, np.int64)
    rank_of_part[perm] = np.arange(128)

    loc = idx - owner * SLICE                 # local row in [0, 384)
    chunk = loc // 128
    part = loc % 128
    # emission key per token: (owner, chunk, round k within its partition
    # bucket, port-interleaved partition rank)
    order0 = np.lexsort((part, chunk, owner))  # group by (owner, chunk, part)
    # round index k within each (owner, chunk, part) bucket
    oo, cc, pp = owner[order0], chunk[order0], part[order0]
    bucket_change = np.r_[True, (oo[1:] != oo[:-1]) | (cc[1:] != cc[:-1]) |
                          (pp[1:] != pp[:-1])]
    bucket_id = np.cumsum(bucket_change) - 1
    pos = np.arange(len(order0))
    k_in_bucket = pos - np.maximum.accumulate(np.where(bucket_change, pos, 0))
    key = np.lexsort((rank_of_part[pp], k_in_bucket, cc, oo))
    order = order0[key]                        # final per-core slot order
    return t_flat, p_flat, idx, order, counts, tiles, cap


def make_in_maps(t, tab, w_proj, fold_dtype=FOLD_DTYPE):
    """Host-side marshalling: route tokens, shard table rows, transpose."""
    tab = np.ascontiguousarray(np.asarray(tab), dtype=np.float32)
    w_proj = np.ascontiguousarray(np.asarray(w_proj), dtype=np.float32)
    t_flat, p_flat, idx, order, counts, tiles, cap = route(t)
    s = cap // 16

    tabT = np.ascontiguousarray(tab.T)                       # [D, SZ]
    wT = np.ascontiguousarray(w_proj.T)                      # [D, D]
    if fold_dtype == "bf16":
        import ml_dtypes
        tabT = tabT.astype(ml_dtypes.bfloat16)
        wT = wT.astype(ml_dtypes.bfloat16)
    # packed [128, KC*D]: [p, kc*D + e] = wT[kc*128 + p, e]
    wT_packed = np.ascontiguousarray(
        wT.reshape(KC, 128, D).transpose(1, 0, 2).reshape(128, KC * D)
    )

    # group layout: fixed sizes, identical across cores
    sizes = []
    left = cap
    while left > 0:
        g = min(GROUP_TOKENS, left)
        sizes.append(g)
        left -= g
    starts = np.cumsum([0] + sizes[:-1]).tolist()

    in_maps = []
    slots_per_core = []
    span_per_core = []
    off = 0
    for c in range(N_CORES):
        n = int(counts[c])
        toks = order[off : off + n]
        off += n
        t_sh = np.zeros(cap, np.int32)
        tp_sh = np.zeros(cap, np.int32)
        t_sh[:n] = t_flat[toks]
        tp_sh[:n] = p_flat[toks]
        # device-visible local row (after clamp) per slot — for group spans
        loc = np.clip(_hash_idx_host(t_sh, tp_sh) - c * SLICE, 0, SLICE - 1)
        spans = [int(loc[g0 : g0 + gs].max() // 128) + 1
                 for g0, gs in zip(starts, sizes)]
        span_per_core.append(spans)
        slots_per_core.append(toks)
        # device layout [128, 2s] int32: cols [0,s) = t, [s,2s) = prev;
        # each a [16, s] block with slot i at [i%16, i//16], replicated
        # across the 8 Q7 partition groups
        blk = np.tile(
            np.concatenate(
                [t_sh.reshape(s, 16).T, tp_sh.reshape(s, 16).T], axis=1
            ),
            (8, 1),
        )
        # tabT slice packed [128, RC*KC*128]:
        # [p, (rc*KC + kc)*128 + m] = tabT[kc*128 + p, c*SLICE + rc*128 + m]
        tabT_sl = (
            tabT[:, c * SLICE : (c + 1) * SLICE]
            .reshape(KC, 128, RC_LOC, 128)
            .transpose(1, 2, 0, 3)
            .reshape(128, RC_LOC * KC * 128)
        )
        in_maps.append(
            {
                "t_sh": np.ascontiguousarray(blk, dtype=np.int32),
                "base": np.full((128, 1), c * SLICE, np.float32),
                "tabT": np.ascontiguousarray(tabT_sl),
                "w_projT": wT_packed,
            }
        )
    # SPMD: identical program everywhere -> max span across cores per group
    groups = tuple(
        (int(g0), int(gs), max(span_per_core[c][i] for c in range(N_CORES)))
        for i, (g0, gs) in enumerate(zip(starts, sizes))
    )
    return in_maps, (slots_per_core, counts, sizes, starts), counts, tiles, groups


def kernel(t, tab, w_proj):
    import ml_dtypes

    in_maps, meta, counts, tiles, groups = make_in_maps(t, tab, w_proj)
    slots_per_core, _, sizes, starts = meta
    nc = build(tiles, bmax=groups)
    res = run_bass_kernel_spmd(nc, in_maps, list(range(N_CORES)))
    out = np.empty((NTOK, D), np.float32)
    for c in range(N_CORES):
        n = int(counts[c])
        r = np.asarray(res.results[c]["out_sh"])
        if r.dtype != ml_dtypes.bfloat16:
            r = r.view(ml_dtypes.bfloat16)
        rows = np.empty((tiles * 128, D), np.float32)
        for g0, gs in zip(starts, sizes):
            blk = r[:, 8 * g0 : 8 * (g0 + gs)]
            if GATHER_SRC == "sbuf":
                # blk[p, c*gs + i] = token (g0+i) element c*128+p
                rows[g0 : g0 + gs] = (
                    blk.reshape(128, 8, gs).transpose(2, 1, 0)
                    .reshape(gs, D).astype(np.float32)
                )
            else:
                # blk[p, j*D + d] = token (g0 + j*128 + p) element d
                rows[g0 : g0 + gs] = (
                    blk.reshape(128, gs // 128, D).transpose(1, 0, 2)
                    .reshape(gs, D).astype(np.float32)
                )
        out[slots_per_core[c]] = rows[:n]
    return out.reshape(B, T, D)
